# revision 1
# baseline (speedup 1.0000x reference)
"""Trainium2 Bass kernel for CustomMultiHeadAttention (B=4, S=1024, D=1024, H=16, Dh=64).

Sharding: 8 cores = (batch b in 0..3) x (parity par in 0..1).
Core (b, par) computes output rows {s : s % 2 == par} of batch b,
grouped into 4 "vblocks" of 128 rows (vblock i' = seq 256*i' + 2*c + par).
K/V are computed for the full sequence on every core (from the full x[b]).
The program is identical on all cores; per-core differences are input data.

Pipeline (all transposed-layout, PE-centric):
  QT = rope(Wq^T x^T), KT = rope(Wk^T x^T)  - rope via permutation-matmul + DVE
  scT[kv,q] = KT_h^T QT_h (2 heads row-packed), exp on ScalarE (scale=1/8),
  causal mask = f16 0/1 multiply on the diagonal 128 cols,
  ctxT/denoms accumulate via lhsT=[V|1], normalize via reciprocal_approx_fast
  + PE broadcast, out = ctxT^T Wo.
"""

import threading

import numpy as np

B, S, D, H, Dh = 4, 1024, 1024, 16, 64
P = 128
N_CORES = 8
NT = D // P  # 8 tiles along d/dout/seq
# scores suffix width per kv-block j (active q-vblocks are a suffix)
NJ = [512, 512, 384, 384, 256, 256, 128, 128]
VS = 65  # V slot width: [V(64) | ones(1)] per head

_cache = {}
_lock = threading.Lock()


def _build_program(taps=False):
    import concourse.bass as bass  # noqa: F401
    import concourse.mybir as mybir
    import concourse.tile as tile
    from concourse import bacc

    dt = mybir.dt
    f16, f32 = dt.float16, dt.float32
    AF = mybir.ActivationFunctionType

    nc = bacc.Bacc("TRN2", target_bir_lowering=False, debug=False,
                   num_devices=N_CORES)

    def ein(name, shape):
        return nc.dram_tensor(name, shape, f16, kind="ExternalInput").ap()

    xt_sh = ein("xt_sh", [P, NT, S])     # x[b]^T, host-transposed
    xqt_sh = ein("xqt_sh", [P, NT, 512])  # xq^T, host-transposed
    w_ext = {n: ein(n, [D, D]) for n in ("wq", "wk", "wv", "wo")}
    bqt_e = nc.dram_tensor("bqt", [P, NT], f32, kind="ExternalInput").ap()
    bkt_e = nc.dram_tensor("bkt", [P, NT], f32, kind="ExternalInput").ap()
    bv_e = ein("bv", [1, D])
    bo_e = ein("bo", [1, D])
    cosq_e = ein("cosq", [P, 512])
    sinq_e = ein("sinq", [P, 512])
    cosk_e = ein("cosk", [P, S])
    sink_e = ein("sink", [P, S])
    mj0_e = ein("mj0", [P, P])
    mj1_e = ein("mj1", [P, P])
    p128_e = ein("p128", [P, P])
    y_sh = nc.dram_tensor("y_sh", [512, D], f16, kind="ExternalOutput").ap()
    tap_ext = {}
    if taps:
        for tn, shape in (("qt", [P, NT, 512]), ("kt", [P, NT, S]),
                          ("v1", [P, NT, H * VS]), ("cn", [P, NT, 512])):
            tap_ext[tn] = nc.dram_tensor("dbg_" + tn, shape, f16,
                                         kind="ExternalOutput").ap()

    with tile.TileContext(nc) as tc:
        from contextlib import ExitStack
        with ExitStack() as ctx:
            big = ctx.enter_context(tc.tile_pool(name="big", bufs=1))

            xT = big.tile([P, NT, S], f16, tag="xT")        # x[b]^T  [din, s]
            xqT = big.tile([P, NT, 512], f16, tag="xqT")    # xq^T    [din, q]
            w_sb = {n: big.tile([P, NT, D], f16, tag=n, name=n + "_sb")
                    for n in w_ext}
            bqt = big.tile([P, NT], f32, tag="bqt")
            bkt = big.tile([P, NT], f32, tag="bkt")
            bv_sb = big.tile([1, D], f16, tag="bv")
            bo_sb = big.tile([1, D], f16, tag="bo")
            qt = big.tile([P, NT, 512], f16, tag="qt")      # rope'd Q^T
            kt = big.tile([P, NT, S], f16, tag="kt")        # rope'd K^T
            v1 = big.tile([P, NT, H * VS], f16, tag="v1")   # [V|1] slots
            cn = big.tile([P, NT, 512], f16, tag="cn")      # normalized ctx^T
            cosq = big.tile([P, 512], f16, tag="cosq")
            sinq = big.tile([P, 512], f16, tag="sinq")
            cosk = big.tile([P, S], f16, tag="cosk")
            sink = big.tile([P, S], f16, tag="sink")
            mj0 = big.tile([P, P], f16, tag="mj0")
            mj1 = big.tile([P, P], f16, tag="mj1")
            p128 = big.tile([P, P], f16, tag="p128")
            ones = big.tile([P, 512], f16, tag="ones")

            # ---- input DMAs ----
            # critical path (sync queue): per-k interleave so Q-proj's
            # k-chain starts as early as possible
            for k in range(NT):
                nc.sync.dma_start(xqT[:, k, :], xqt_sh[:, k, :])
                nc.sync.dma_start(w_sb["wq"][:, k, :],
                                  w_ext["wq"][P * k:P * (k + 1), :])
                if k == 3:
                    for t, e in ((p128, p128_e), (cosq, cosq_e),
                                 (sinq, sinq_e), (bqt, bqt_e)):
                        nc.sync.dma_start(t[:], e[:])
            for k in range(NT):
                nc.sync.dma_start(xT[:, k, :], xt_sh[:, k, :])
                nc.sync.dma_start(w_sb["wk"][:, k, :],
                                  w_ext["wk"][P * k:P * (k + 1), :])
            for t, e in ((cosk, cosk_e), (sink, sink_e), (bkt, bkt_e)):
                nc.sync.dma_start(t[:], e[:])
            # bulk weights on the gpsimd queue, in parallel
            for k in range(NT):
                nc.gpsimd.dma_start(w_sb["wv"][:, k, :],
                                    w_ext["wv"][P * k:P * (k + 1), :])
            for t, e in ((bv_sb, bv_e), (mj0, mj0_e), (mj1, mj1_e)):
                nc.gpsimd.dma_start(t[:], e[:])
            for k in range(NT):
                nc.gpsimd.dma_start(w_sb["wo"][:, k, :],
                                    w_ext["wo"][P * k:P * (k + 1), :])
            nc.gpsimd.dma_start(bo_sb[:], bo_e[:])
            nc.any.memset(ones[:], 1.0)
            # ones columns of the V slots (col 64 of each 65-wide slot)
            v1r = v1.rearrange("p t (h c) -> p t h c", c=VS)
            for t in range(NT):
                nc.any.memset(v1r[:, t, :, 64:65], 1.0)

            # ---- projections + rope ----
            with tc.tile_pool(name="pp", bufs=2, space="PSUM") as pp, \
                 tc.tile_pool(name="sc", bufs=4) as sc:

                def rope_block(dst, w_name, bias_col, rhs, cos_ap, sin_ap,
                               dst_sl):
                    # dst [128, 512] <- rope(W^T @ x^T + b) for one dout tile
                    ps = pp.tile([P, 512], f32, tag="ps", name="ps")
                    for k in range(NT):
                        nc.tensor.matmul(ps[:], w_sb[w_name][:, k, dst_sl],
                                         rhs(k), start=(k == 0),
                                         stop=(k == NT - 1))
                    # psum->sbuf f16 with fused per-partition bias (DVE)
                    raw = sc.tile([P, 512], f16, tag="raw", name="raw")
                    nc.vector.tensor_scalar_add(raw[:], ps[:], bias_col)
                    pq = pp.tile([P, 512], f32, tag="pq", name="pq")
                    nc.tensor.matmul(pq[:], p128[:], raw[:],
                                     start=True, stop=True)
                    t1 = sc.tile([P, 512], f16, tag="t1", name="t1")
                    nc.vector.tensor_mul(t1[:], raw[:], cos_ap)
                    t2 = sc.tile([P, 512], f16, tag="t2", name="t2")
                    nc.vector.tensor_mul(t2[:], pq[:], sin_ap)
                    nc.vector.tensor_add(dst, t1[:], t2[:])

                for t in range(NT):
                    dst_sl = slice(P * t, P * (t + 1))
                    rope_block(qt[:, t, :], "wq", bqt[:, t:t + 1],
                               lambda k: xqT[:, k, :], cosq[:], sinq[:],
                               dst_sl)
                    for n in range(2):
                        csl = slice(512 * n, 512 * (n + 1))
                        rope_block(kt[:, t, csl], "wk", bkt[:, t:t + 1],
                                   lambda k, csl=csl: xT[:, k, csl],
                                   cosk[:, csl], sink[:, csl], dst_sl)
                    # V tile t (s-tile): natural [s, dout] into 65-wide slots
                    for n in range(2):
                        csl = slice(512 * n, 512 * (n + 1))
                        vp = pp.tile([P, 512], f32, tag="vp", name="vp")
                        for k in range(NT):
                            nc.tensor.matmul(vp[:], xT[:, k, dst_sl],
                                             w_sb["wv"][:, k, csl],
                                             start=(k == 0), stop=False)
                        nc.tensor.matmul(vp[:], ones[0:1, 0:P],
                                         bv_sb[0:1, csl],
                                         start=False, stop=True)
                        nc.vector.tensor_copy(
                            v1r[:, t, 8 * n:8 * n + 8, 0:64],
                            vp.rearrange("p (h c) -> p h c", c=64))

            # ---- attention (per head pair p: heads 2p, 2p+1) ----
            with tc.tile_pool(name="scp", bufs=2, space="PSUM") as scp, \
                 tc.tile_pool(name="cxp", bufs=1, space="PSUM") as cxp, \
                 tc.tile_pool(name="dnp", bufs=2, space="PSUM") as dnp, \
                 tc.tile_pool(name="ep", bufs=3) as ep, \
                 tc.tile_pool(name="npl", bufs=2) as npl:
                for p in range(NT):
                    cx0 = cxp.tile([64, 512], f32, tag="cx0", name="cx0")
                    cx1 = cxp.tile([P, 512], f32, tag="cx1", name="cx1")
                    d0 = dnp.tile([1, 512], f32, tag="d", name="d0")
                    d1 = dnp.tile([1, 512], f32, tag="d", name="d1")
                    h0, h1 = 2 * p, 2 * p + 1
                    es = {}

                    def emit_scores(j):
                        N = NJ[j]
                        co = 512 - N
                        s_ps = scp.tile([P, 1024], f32, tag="s",
                                        name=f"s{p}_{j}")
                        for h in range(2):
                            rsl = slice(64 * h, 64 * (h + 1))
                            nc.tensor.matmul(s_ps[:, 512 * h:512 * h + N],
                                             kt[rsl, p, P * j:P * (j + 1)],
                                             qt[rsl, p, co:512],
                                             start=True, stop=True,
                                             skip_group_check=True)
                        e = ep.tile([P, 1024], f16, tag="e",
                                    name=f"e{p}_{j}")
                        sv = s_ps.rearrange("q (a n) -> q a n", a=2)
                        ev = e.rearrange("q (a n) -> q a n", a=2)
                        nc.scalar.activation(ev[:, :, 0:N], sv[:, :, 0:N],
                                             AF.Exp, scale=0.125)
                        mj = mj0 if j % 2 == 0 else mj1
                        nc.vector.tensor_mul(e[:, 0:P], e[:, 0:P], mj[:])
                        nc.vector.tensor_mul(e[:, 512:512 + P],
                                             e[:, 512:512 + P], mj[:])
                        es[j] = e

                    def emit_ctx(j):
                        N = NJ[j]
                        co = 512 - N
                        e = es.pop(j)
                        st, sp = (j == 0), (j == NT - 1)
                        nc.tensor.matmul(cx0[:, co:512],
                                         v1[:, j, VS * h0:VS * h0 + 64],
                                         e[:, 0:N], start=st, stop=sp)
                        nc.tensor.matmul(cx1[64:P, co:512],
                                         v1[:, j, VS * h1:VS * h1 + 64],
                                         e[:, 512:512 + N],
                                         start=st, stop=sp)
                        nc.tensor.matmul(d0[0:1, co:512], ones[:, 0:1],
                                         e[:, 0:N], start=st, stop=sp)
                        nc.tensor.matmul(d1[0:1, co:512], ones[:, 0:1],
                                         e[:, 512:512 + N],
                                         start=st, stop=sp)

                    # depth-2 software pipeline: scores run ahead of ctx
                    for j in range(NT + 2):
                        if j < NT:
                            emit_scores(j)
                        if j >= 2:
                            emit_ctx(j - 2)

                    # normalize: recip of denoms, PE-broadcast, multiply
                    r0 = npl.tile([1, 512], f32, tag="r", name="r0")
                    nc.vector.reciprocal_approx_fast(r0[:], d0[:])
                    r0h = npl.tile([1, 512], f16, tag="rh", name="r0h")
                    nc.vector.tensor_copy(r0h[:], r0[:])
                    r1 = npl.tile([1, 512], f32, tag="r", name="r1")
                    nc.vector.reciprocal_approx_fast(r1[:], d1[:])
                    r1h = npl.tile([1, 512], f16, tag="rh", name="r1h")
                    nc.vector.tensor_copy(r1h[:], r1[:])
                    rb = scp.tile([P, 1024], f32, tag="s", name="rb")
                    nc.tensor.matmul(rb[0:64, 0:512], ones[0:1, 0:64],
                                     r0h[:], start=True, stop=True,
                                     skip_group_check=True)
                    nc.tensor.matmul(rb[64:P, 0:512], ones[0:1, 0:64],
                                     r1h[:], start=True, stop=True,
                                     tile_position=(0, 64),
                                     skip_group_check=True)
                    rbs = npl.tile([P, 512], f32, tag="rbs", name="rbs")
                    nc.vector.tensor_copy(rbs[:], rb[:, 0:512])
                    nc.vector.tensor_mul(cn[0:64, p, :], cx0[0:64, :],
                                         rbs[0:64, :])
                    nc.vector.tensor_mul(cn[64:P, p, :], cx1[64:P, :],
                                         rbs[64:P, :])

            if taps:
                for tn, tile_ap in (("qt", qt), ("kt", kt), ("v1", v1),
                                    ("cn", cn)):
                    nc.sync.dma_start(tap_ext[tn][:], tile_ap[:])

            # ---- output projection ----
            with tc.tile_pool(name="op", bufs=4, space="PSUM") as op, \
                 tc.tile_pool(name="ob", bufs=4) as ob:
                for i in range(4):
                    for n in range(2):
                        csl = slice(512 * n, 512 * (n + 1))
                        yp = op.tile([P, 512], f32, tag="yp", name="yp")
                        for t in range(NT):
                            nc.tensor.matmul(yp[:], cn[:, t, P * i:P * (i + 1)],
                                             w_sb["wo"][:, t, csl],
                                             start=(t == 0), stop=False)
                        nc.tensor.matmul(yp[:], ones[0:1, 0:P],
                                         bo_sb[0:1, csl],
                                         start=False, stop=True)
                        ys = ob.tile([P, 512], f16, tag="ys", name="ys")
                        nc.vector.tensor_copy(ys[:], yp[:])
                        nc.sync.dma_start(y_sh[P * i:P * (i + 1), csl], ys[:])

    nc.compile()
    return nc


def _host_tables():
    # RoPE tables, computed in float32 to match the reference's jnp path.
    pos = np.arange(S, dtype=np.float32)
    inv = np.exp(np.arange(0, Dh, 2, dtype=np.float32)
                 * np.float32(-np.log(10000.0) / Dh))          # [32]
    ang = pos[:, None] * inv[None, :]                          # [S, 32]
    sin = np.sin(ang).astype(np.float32)
    cos = np.cos(ang).astype(np.float32)
    # per-partition pattern for [2 heads x 64, s] transposed layout
    dd = np.arange(P) % Dh
    cosP = np.empty((P, S), np.float32)
    sinP = np.empty((P, S), np.float32)
    lo = dd < 32
    cosP[lo] = cos[:, dd[lo]].T
    sinP[lo] = -sin[:, dd[lo]].T
    cosP[~lo] = cos[:, dd[~lo] - 32].T
    sinP[~lo] = sin[:, dd[~lo] - 32].T
    return cosP.astype(np.float16), sinP.astype(np.float16)


def _perm128():
    p = np.zeros((P, P), np.float16)
    i = np.arange(P)
    p[i, i ^ 32] = np.float16(1.0)
    return p


def _tile_T(a):
    # [rows, D] -> [P, NT, rows]: partition-tiled transpose for SBUF layout
    rows = a.shape[0]
    return np.ascontiguousarray(a.T.reshape(NT, P, rows).transpose(1, 0, 2))


def make_in_maps(x, Wq, bq, Wk, bk, Wv, bv, Wo, bo):
    x = np.asarray(x, np.float16)
    shared = {
        "wq": np.ascontiguousarray(np.asarray(Wq, np.float16)),
        "wk": np.ascontiguousarray(np.asarray(Wk, np.float16)),
        "wv": np.ascontiguousarray(np.asarray(Wv, np.float16)),
        "wo": np.ascontiguousarray(np.asarray(Wo, np.float16)),
        "bqt": np.ascontiguousarray(
            np.asarray(bq, np.float16).astype(np.float32).reshape(NT, P).T),
        "bkt": np.ascontiguousarray(
            np.asarray(bk, np.float16).astype(np.float32).reshape(NT, P).T),
        "bv": np.asarray(bv, np.float16).reshape(1, D),
        "bo": np.asarray(bo, np.float16).reshape(1, D),
        "p128": _perm128(),
    }
    cosP, sinP = _host_tables()
    shared["cosk"] = cosP
    shared["sink"] = sinP

    in_maps = []
    for core in range(N_CORES):
        b, par = core // 2, core % 2
        xb = x[b]                                   # [1024, 1024]
        x4 = xb.reshape(4, P, 2, D)
        cos4 = cosP.reshape(P, 4, P, 2)
        sin4 = sinP.reshape(P, 4, P, 2)
        r = np.arange(P)[:, None]
        cc2 = 2 * np.arange(P)[None, :] + par
        xq = x4[:, :, par, :].reshape(512, D)
        m = {
            "xt_sh": _tile_T(xb),
            "xqt_sh": _tile_T(xq),
            "cosq": np.ascontiguousarray(cos4[:, :, :, par].reshape(P, 512)),
            "sinq": np.ascontiguousarray(sin4[:, :, :, par].reshape(P, 512)),
            "mj0": (r <= cc2).astype(np.float16),
            "mj1": (r + P <= cc2).astype(np.float16),
        }
        m.update(shared)
        in_maps.append(m)
    return in_maps


def kernel(x, Wq, bq, Wk, bk, Wv, bv, Wo, bo):
    from concourse.bass_utils import run_bass_kernel_spmd

    with _lock:
        if "nc" not in _cache:
            _cache["nc"] = _build_program()
    nc = _cache["nc"]

    in_maps = make_in_maps(x, Wq, bq, Wk, bk, Wv, bv, Wo, bo)
    res = run_bass_kernel_spmd(nc, in_maps, list(range(N_CORES)))

    out = np.empty((B, S, D), np.float16)
    o4 = out.reshape(B, 4, P, 2, D)
    for core in range(N_CORES):
        b, par = core // 2, core % 2
        o4[b, :, :, par, :] = res.results[core]["y_sh"].reshape(4, P, D)
    return out



# revision 18
# speedup vs baseline: 1.5458x; 1.5458x over previous
"""Trainium2 Bass kernel for CustomMultiHeadAttention (B=4, S=1024, D=1024, H=16, Dh=64).

Sharding: 8 cores = (batch b in 0..3) x (head-half hh in 0..1).
Core (b, hh) computes heads 8*hh..8*hh+7 of batch b over the FULL
sequence (natural q order), producing a partial output
y_part = ctx_half @ Wo[512*hh:512*hh+512, :]; the host sums the two
partials per batch (and adds bo).  This halves projection FLOPs and
weight DMA versus data-parallel-over-queries.

Pipeline (transposed layout, PE-centric):
  QT = rope(Wq^T x^T), KT = rope(Wk^T x^T)  - rope via permutation-matmul + DVE
  per head pair p (4) x q-chunk n (2): scT[kv,q] = KT_h^T QT_h,
  exp on ScalarE (scale=1/8), causal mask on the diagonal 128-col block,
  ctx accumulates with lhsT = [V_h0|1|V_h1] slots: h0 gets a free
  denominator row (M=65), h1's denominator is one M=1 matmul into the
  unused row 0 of its ctx PSUM bank.  Normalization reciprocals are
  partition-broadcast on GpSimd (no PE broadcast matmul), then
  cn = ctx * recip on DVE; out = cn^T Wo_half.
"""

import threading

import numpy as np

B, S, D, H, Dh = 4, 1024, 1024, 16, 64
P = 128
N_CORES = 8
KT = 8    # k (din) tiles
HT = 4    # dout tiles per core (8 heads = 512 dims)
ST = 8    # s tiles
VS2 = 130  # V pair slot: [V_h0(64) | 1 | V_h1(64) | 1]

_cache = {}
_lock = threading.Lock()


def _build_program(taps=False):
    import concourse.bass as bass  # noqa: F401
    import concourse.mybir as mybir
    import concourse.tile as tile
    from concourse import bacc

    dt = mybir.dt
    f16, f32 = dt.float16, dt.float32
    AF = mybir.ActivationFunctionType

    nc = bacc.Bacc("TRN2", target_bir_lowering=False, debug=False,
                   num_devices=N_CORES)

    def ein(name, shape):
        return nc.dram_tensor(name, shape, f16, kind="ExternalInput").ap()

    xt_sh = ein("xt_sh", [P, KT, S])      # x[b]^T, host-transposed
    wq_e = ein("wq", [D, 512])            # Wq[:, half-cols]
    wk_e = ein("wk", [D, 512])
    wv_e = ein("wv", [D, 512])
    wo_e = ein("wo", [512, D])            # Wo[half-rows, :]
    bqt_e = nc.dram_tensor("bqt", [P, HT], f32, kind="ExternalInput").ap()
    bkt_e = nc.dram_tensor("bkt", [P, HT], f32, kind="ExternalInput").ap()
    bv_e = ein("bv", [1, 512])
    cos_e = ein("cosT", [P, S])
    sin_e = ein("sinT", [P, S])
    mj_e = ein("mj", [P, P])
    p128_e = ein("p128", [P, P])
    y_sh = nc.dram_tensor("y_sh", [S, D], f16, kind="ExternalOutput").ap()
    tap_ext = {}
    if taps:
        for tn, shape in (("qt", [P, HT, S]), ("kt", [P, HT, S]),
                          ("v1", [P, ST, HT * VS2]), ("cn", [P, HT, S])):
            tap_ext[tn] = nc.dram_tensor("dbg_" + tn, shape, f16,
                                         kind="ExternalOutput").ap()
        tap_ext["nrm"] = nc.dram_tensor("dbg_nrm", [64, 8, 1024], f32,
                                        kind="ExternalOutput").ap()
        tap_ext["den"] = nc.dram_tensor("dbg_den", [8, 1024], f32,
                                        kind="ExternalOutput").ap()
        tap_ext["r01"] = nc.dram_tensor("dbg_r01", [8, 1024], f32,
                                        kind="ExternalOutput").ap()

    with tile.TileContext(nc) as tc:
        from contextlib import ExitStack
        with ExitStack() as ctx:
            big = ctx.enter_context(tc.tile_pool(name="big", bufs=1))

            xT = big.tile([P, KT, S], f16, tag="xT")       # x^T  [din, s]
            wq = big.tile([P, KT, 512], f16, tag="wq")
            wk = big.tile([P, KT, 512], f16, tag="wk")
            wv = big.tile([P, KT, 512], f16, tag="wv")
            wo = big.tile([P, HT, D], f16, tag="wo")
            bqt = big.tile([P, HT], f32, tag="bqt")
            bkt = big.tile([P, HT], f32, tag="bkt")
            bv_sb = big.tile([1, 512], f16, tag="bv")
            qt = big.tile([P, HT, S], f16, tag="qt")       # rope'd Q^T
            kt = big.tile([P, HT, S], f16, tag="kt")       # rope'd K^T
            v1 = big.tile([P, ST, HT * VS2], f16, tag="v1")
            cn = big.tile([P, HT, S], f16, tag="cn")       # normalized ctx^T
            cosT = big.tile([P, S], f16, tag="cosT")
            sinT = big.tile([P, S], f16, tag="sinT")
            mj = big.tile([P, P], f16, tag="mj")
            p128 = big.tile([P, P], f16, tag="p128")
            ones = big.tile([P, P], f16, tag="ones")

            # ---- input DMAs, priority order on the sync queue ----
            for k in range(KT):
                nc.sync.dma_start(xT[:, k, :], xt_sh[:, k, :])
                nc.sync.dma_start(wq[:, k, :], wq_e[P * k:P * (k + 1), :])
                if k == 2:
                    for t, e in ((p128, p128_e), (cosT, cos_e),
                                 (sinT, sin_e), (bqt, bqt_e)):
                        nc.sync.dma_start(t[:], e[:])
            for k in range(KT):
                nc.sync.dma_start(wk[:, k, :], wk_e[P * k:P * (k + 1), :])
            nc.sync.dma_start(bkt[:], bkt_e[:])
            for k in range(KT):
                nc.sync.dma_start(wv[:, k, :], wv_e[P * k:P * (k + 1), :])
            nc.sync.dma_start(bv_sb[:], bv_e[:])
            for t in range(HT):
                nc.sync.dma_start(wo[:, t, :], wo_e[P * t:P * (t + 1), :])
            # small/late tensors on the gpsimd queue
            nc.gpsimd.dma_start(mj[:], mj_e[:])
            nc.any.memset(ones[:], 1.0)
            v1r = v1.rearrange("p t (pr c) -> p t pr c", c=VS2)
            nc.any.memset(v1r[:, :, :, 64:65], 1.0)
            nc.any.memset(v1r[:, :, :, 129:130], 1.0)

            # ---- projections + rope ----
            with tc.tile_pool(name="pp", bufs=2, space="PSUM") as pp, \
                 tc.tile_pool(name="sc", bufs=4) as sc:

                def rope_block(dst, w_sb, bias_col):
                    # dst [128, S] <- rope(W^T @ x^T + b) for one dout tile
                    # (matmuls split in 512-col halves: one PSUM bank each)
                    ps = pp.tile([P, S], f32, tag="ps", name="ps")
                    for c in range(2):
                        csl = slice(512 * c, 512 * (c + 1))
                        for k in range(KT):
                            nc.tensor.matmul(ps[:, csl], w_sb(k),
                                             xT[:, k, csl],
                                             start=(k == 0),
                                             stop=(k == KT - 1),
                                             skip_group_check=True)
                    raw = sc.tile([P, S], f16, tag="raw", name="raw")
                    nc.vector.tensor_scalar_add(raw[:], ps[:], bias_col)
                    pq = pp.tile([P, S], f32, tag="pq", name="pq")
                    for c in range(2):
                        csl = slice(512 * c, 512 * (c + 1))
                        nc.tensor.matmul(pq[:, csl], p128[:], raw[:, csl],
                                         start=True, stop=True,
                                         skip_group_check=True)
                    t1 = sc.tile([P, S], f16, tag="t1", name="t1")
                    nc.vector.tensor_mul(t1[:], raw[:], cosT[:])
                    t2 = sc.tile([P, S], f16, tag="t2", name="t2")
                    nc.vector.tensor_mul(t2[:], pq[:], sinT[:])
                    nc.vector.tensor_add(dst, t1[:], t2[:])

                for t in range(HT):
                    dst_sl = slice(P * t, P * (t + 1))
                    rope_block(qt[:, t, :],
                               lambda k, s=dst_sl: wq[:, k, s],
                               bqt[:, t:t + 1])
                    rope_block(kt[:, t, :],
                               lambda k, s=dst_sl: wk[:, k, s],
                               bkt[:, t:t + 1])
                # V: per s-tile, natural [s, dout] into 129-wide pair slots
                for t in range(ST):
                    ssl = slice(P * t, P * (t + 1))
                    vp = pp.tile([P, S], f32, tag="ps", name="vp")
                    for k in range(KT):
                        nc.tensor.matmul(vp[:, 0:512], xT[:, k, ssl],
                                         wv[:, k, :],
                                         start=(k == 0), stop=False)
                    nc.tensor.matmul(vp[:, 0:512], ones[0:1, 0:P],
                                     bv_sb[0:1, :], start=False, stop=True)
                    vpr = vp[:, 0:512].rearrange("p (pr two c) -> p pr two c",
                                                 two=2, c=64)
                    nc.vector.tensor_copy(v1r[:, t, :, 0:64],
                                          vpr[:, :, 0, :])
                    nc.vector.tensor_copy(v1r[:, t, :, 65:129],
                                          vpr[:, :, 1, :])

            # ---- attention (head pair p, q-chunk n of 512) ----
            with tc.tile_pool(name="scp", bufs=2, space="PSUM") as scp, \
                 tc.tile_pool(name="cxp", bufs=2, space="PSUM") as cxp, \
                 tc.tile_pool(name="ep", bufs=3) as ep, \
                 tc.tile_pool(name="npl", bufs=2) as npl:
                for p in range(HT):
                    for n in range(2):
                        js = list(range(4 * n + 4))
                        qlo = 512 * n
                        cx0 = cxp.tile([65, 512], f32, tag="cx0", name="cx0")
                        cx1 = cxp.tile([65, 512], f32, tag="cx1", name="cx1")
                        es = {}

                        def emit_scores(j, n=n, p=p, qlo=qlo, es=es):
                            N = 512 if j < 4 * n else 512 - P * (j - 4 * n)
                            co = 512 - N
                            s_ps = scp.tile([P, 1024], f32, tag="s",
                                            name=f"s{p}_{n}_{j}")
                            for h in range(2):
                                rsl = slice(64 * h, 64 * (h + 1))
                                nc.tensor.matmul(
                                    s_ps[:, 512 * h:512 * h + N],
                                    kt[rsl, p, P * j:P * (j + 1)],
                                    qt[rsl, p, qlo + co:qlo + 512],
                                    start=True, stop=True,
                                    skip_group_check=True)
                            e = ep.tile([P, 1024], f16, tag="e",
                                        name=f"e{p}_{n}_{j}")
                            sv = s_ps.rearrange("q (a m) -> q a m", a=2)
                            ev = e.rearrange("q (a m) -> q a m", a=2)
                            nc.scalar.activation(ev[:, :, 0:N], sv[:, :, 0:N],
                                                 AF.Exp, scale=0.125)
                            if j >= 4 * n:
                                nc.vector.tensor_mul(e[:, 0:P], e[:, 0:P],
                                                     mj[:])
                                nc.vector.tensor_mul(e[:, 512:512 + P],
                                                     e[:, 512:512 + P], mj[:])
                            es[j] = e

                        def emit_ctx(j, n=n, p=p, js=js, es=es,
                                     cx0=cx0, cx1=cx1):
                            N = 512 if j < 4 * n else 512 - P * (j - 4 * n)
                            co = 512 - N
                            e = es.pop(j)
                            st, sp = (j == 0), (j == js[-1])
                            nc.tensor.matmul(cx0[0:65, co:512],
                                             v1r[:, j, p, 0:65],
                                             e[:, 0:N], start=st, stop=sp)
                            nc.tensor.matmul(cx1[0:65, co:512],
                                             v1r[:, j, p, 65:130],
                                             e[:, 512:512 + N],
                                             start=st, stop=sp)

                        for step in range(len(js) + 2):
                            if step < len(js):
                                emit_scores(js[step])
                            if step >= 2:
                                emit_ctx(js[step - 2])

                        # normalize: recip of both free denominator rows,
                        # one GpSimd broadcast to partitions 0:64, DVE muls;
                        # odd head's rows shift 0:64 -> 64:128 via SB->SB DMA
                        dn = npl.tile([1, 1024], f32, tag="dn", name="dn")
                        nc.vector.tensor_copy(dn[:, 0:512], cx0[64:65, :])
                        nc.vector.tensor_copy(dn[:, 512:1024], cx1[64:65, :])
                        r01 = npl.tile([1, 1024], f32, tag="r", name="r01")
                        nc.vector.reciprocal_approx_fast(r01[:], dn[:])
                        nrm = npl.tile([64, 1024], f32, tag="nrm", name="nrm")
                        nc.gpsimd.partition_broadcast(nrm[:], r01[:])
                        if taps:
                            g = 2 * p + n
                            dcp = npl.tile([1, 1024], f32, tag="dcp",
                                           name="dcp")
                            nc.vector.tensor_copy(dcp[:, 0:512],
                                                  cx0[64:65, :])
                            nc.vector.tensor_copy(dcp[:, 512:1024],
                                                  cx1[64:65, :])
                            nc.sync.dma_start(tap_ext["den"][g:g + 1, :],
                                              dcp[:])
                            nc.sync.dma_start(tap_ext["r01"][g:g + 1, :],
                                              r01[:])
                            nc.sync.dma_start(tap_ext["nrm"][:, g, :],
                                              nrm[:])
                        qsl = slice(qlo, qlo + 512)
                        nc.vector.tensor_mul(cn[0:64, p, qsl], cx0[0:64, :],
                                             nrm[:, 0:512])
                        stg = npl.tile([64, 512], f16, tag="stg", name="stg")
                        nc.vector.tensor_mul(stg[:], cx1[0:64, :],
                                             nrm[:, 512:1024])
                        nc.sync.dma_start(cn[64:P, p, qsl], stg[:])

            if taps:
                for tn, tile_ap in (("qt", qt), ("kt", kt), ("v1", v1),
                                    ("cn", cn)):
                    nc.sync.dma_start(tap_ext[tn][:], tile_ap[:])

            # ---- output projection (partial: contract this core's 512) ----
            with tc.tile_pool(name="op", bufs=4, space="PSUM") as op, \
                 tc.tile_pool(name="ob", bufs=4) as ob:
                for i in range(8):
                    for m in range(2):
                        csl = slice(512 * m, 512 * (m + 1))
                        yp = op.tile([P, 512], f32, tag="yp", name="yp")
                        for t in range(HT):
                            nc.tensor.matmul(yp[:], cn[:, t, P * i:P * (i + 1)],
                                             wo[:, t, csl],
                                             start=(t == 0), stop=(t == HT - 1))
                        ys = ob.tile([P, 512], f16, tag="ys", name="ys")
                        nc.vector.tensor_copy(ys[:], yp[:])
                        nc.sync.dma_start(y_sh[P * i:P * (i + 1), csl], ys[:])

    nc.compile()
    return nc


def _host_tables():
    # RoPE tables, computed in float32 to match the reference's jnp path.
    pos = np.arange(S, dtype=np.float32)
    inv = np.exp(np.arange(0, Dh, 2, dtype=np.float32)
                 * np.float32(-np.log(10000.0) / Dh))          # [32]
    ang = pos[:, None] * inv[None, :]                          # [S, 32]
    sin = np.sin(ang).astype(np.float32)
    cos = np.cos(ang).astype(np.float32)
    # per-partition pattern for [2 heads x 64, s] transposed layout
    dd = np.arange(P) % Dh
    cosP = np.empty((P, S), np.float32)
    sinP = np.empty((P, S), np.float32)
    lo = dd < 32
    cosP[lo] = cos[:, dd[lo]].T
    sinP[lo] = -sin[:, dd[lo]].T
    cosP[~lo] = cos[:, dd[~lo] - 32].T
    sinP[~lo] = sin[:, dd[~lo] - 32].T
    return cosP.astype(np.float16), sinP.astype(np.float16)


def _perm128():
    p = np.zeros((P, P), np.float16)
    i = np.arange(P)
    p[i, i ^ 32] = np.float16(1.0)
    return p


def _tile_T(a):
    # [rows, D] -> [P, KT, rows]: partition-tiled transpose for SBUF layout
    rows = a.shape[0]
    return np.ascontiguousarray(a.T.reshape(KT, P, rows).transpose(1, 0, 2))


def make_in_maps(x, Wq, bq, Wk, bk, Wv, bv, Wo, bo):
    x = np.asarray(x, np.float16)
    Wq = np.asarray(Wq, np.float16)
    Wk = np.asarray(Wk, np.float16)
    Wv = np.asarray(Wv, np.float16)
    Wo = np.asarray(Wo, np.float16)
    cosP, sinP = _host_tables()
    r = np.arange(P)[:, None]
    c = np.arange(P)[None, :]
    shared = {
        "cosT": cosP,
        "sinT": sinP,
        "mj": (r <= c).astype(np.float16),
        "p128": _perm128(),
    }

    in_maps = []
    for core in range(N_CORES):
        b, hh = core // 2, core % 2
        hsl = slice(512 * hh, 512 * hh + 512)
        m = {
            "xt_sh": _tile_T(x[b]),
            "wq": np.ascontiguousarray(Wq[:, hsl]),
            "wk": np.ascontiguousarray(Wk[:, hsl]),
            "wv": np.ascontiguousarray(Wv[:, hsl]),
            "wo": np.ascontiguousarray(Wo[hsl, :]),
            "bqt": np.ascontiguousarray(
                np.asarray(bq[hsl], np.float16).astype(np.float32)
                .reshape(HT, P).T),
            "bkt": np.ascontiguousarray(
                np.asarray(bk[hsl], np.float16).astype(np.float32)
                .reshape(HT, P).T),
            "bv": np.asarray(bv[hsl], np.float16).reshape(1, 512),
        }
        m.update(shared)
        in_maps.append(m)
    return in_maps


def kernel(x, Wq, bq, Wk, bk, Wv, bv, Wo, bo):
    from concourse.bass_utils import run_bass_kernel_spmd

    with _lock:
        if "nc" not in _cache:
            _cache["nc"] = _build_program()
    nc = _cache["nc"]

    in_maps = make_in_maps(x, Wq, bq, Wk, bk, Wv, bv, Wo, bo)
    res = run_bass_kernel_spmd(nc, in_maps, list(range(N_CORES)))

    bo32 = np.asarray(bo, np.float16).astype(np.float32)
    out = np.empty((B, S, D), np.float16)
    for b in range(B):
        y0 = res.results[2 * b]["y_sh"].astype(np.float32)
        y1 = res.results[2 * b + 1]["y_sh"].astype(np.float32)
        out[b] = (y0 + y1 + bo32).astype(np.float16)
    return out
